# revision 3
# baseline (speedup 1.0000x reference)
"""16-qubit quantum state-vector simulator on 8 Trainium2 NeuronCores.

Circuit (from the reference nn module): per-qubit RY encoder (3 summed
angles/qubit), 120 CRZ gates (all diagonal -> one phase field), then 2
trainable layers of (RX,RY,RZ per wire + CNOT ring). Batch 8 -> one batch
element per core; the full 2^16 complex state lives in SBUF as two
[128, 512] fp32 planes (partition bits = wires 0..6, free bits = 7..15).

Device pipeline per core:
  phi-field matmul (K=9) -> range-reduced Sin/Cos -> 4 matmul stages, each an
  arbitrary operator on 8 bits (7 partition bits + free-MSB, 2x2 blocks of
  128x128 stationaries over half-column groups) with PE transposes between.
  The CNOT-ring bit-permutations M1 factor exactly into these stages:
  M1 = T_low * T_up' * diag(K,S) with the rank-1 cross factors realized as
  conditional column-XOR access patterns (out-AP XOR at stage 2, reversed
  rhs at stage 3). The final ring M2 + residual frame = host-side gather.
"""
import sys, os
import numpy as np

sys.path.insert(0, "/opt/trn_rl_repo")

import concourse.bass as bass
import concourse.mybir as mybir
import concourse.tile as tile
from concourse.bass_utils import run_bass_kernel_spmd

F32 = mybir.dt.float32
F32R = mybir.dt.float32r
I32 = mybir.dt.int32
ACT = mybir.ActivationFunctionType
ALU = mybir.AluOpType
PI = float(np.pi)
N = 16

# ----------------------------------------------------------------------------
# harness compatibility patches (this container's walrus allows only one
# sync-wait per instruction; TileContext's tail drain carries ~27)
# ----------------------------------------------------------------------------
def _install_patches():
    import json as _json
    from concourse.vector_clock import ScopedClock, VectorClock
    from concourse.tile_scheduler import N_PROCS

    def _patched_drain_and_barrier(self, tick_clock, wait_clock):
        gc = tick_clock.global_clock
        vals = [gc[p] for p in range(N_PROCS)]
        for start in range(0, N_PROCS, 1):
            partial = VectorClock(
                [vals[p] if p == start else 0 for p in range(N_PROCS)]
            )
            nop = self.nc.sync.nop(nofuse=True)
            wait_clock.add_sem_waits(nop.ins, ScopedClock({None: partial}))
        self.nc.sync.drain()
        self.nc.all_engine_barrier()
        assert self.sems is not None
        popped = self.nc._tile_sem_poison_stack.pop()
        assert popped is self._sem_poison
        self.nc.clear_and_free_semaphores(list(self.sems.allocated().values()))
        self.nc.all_engine_barrier()

    tile.TileContext._drain_and_barrier = _patched_drain_and_barrier

    if getattr(bass.Bass, "_ant_wait_split_installed", False):
        return
    _orig = bass.Bass.to_json_bytes

    def _patched_json(self):
        j = _json.loads(_orig(self))
        ctr = 0
        for func in j.get("functions", []):
            for blk in func.get("blocks", []):
                out = []
                for ins in blk.get("instructions", []):
                    si = ins.get("sync_info")
                    waits = (si or {}).get("on_wait") or []
                    if len(waits) > 1:
                        head, tail = waits[:-1], waits[-1:]
                        for w in head:
                            ctr += 1
                            out.append({
                                "engine": ins["engine"], "ins": [], "outs": [],
                                "name": f"EVW-{ctr}-{ins['name']}",
                                "opcode": "EventSemaphore",
                                "sync_info": {"on_update": [], "on_wait": [w]},
                            })
                        si["on_wait"] = tail
                    out.append(ins)
                blk["instructions"] = out
        return _json.dumps(j).encode()

    bass.Bass.to_json_bytes = _patched_json
    bass.Bass._ant_wait_split_installed = True

_install_patches()

# ----------------------------------------------------------------------------
# F2 linear algebra
# ----------------------------------------------------------------------------
def _mm(*Ms):
    R = Ms[0]
    for M in Ms[1:]:
        R = (R @ M) % 2
    return R.astype(np.uint8)

def _inv(A):
    n = A.shape[0]
    M = np.concatenate([A.copy() % 2, np.eye(n, dtype=np.uint8)], axis=1)
    r = 0
    for c in range(n):
        piv = next(i for i in range(r, n) if M[i, c])
        M[[r, piv]] = M[[piv, r]]
        for i in range(n):
            if i != r and M[i, c]:
                M[i] ^= M[r]
        r += 1
    return M[:, n:]

def _perm_mat(perm, n=16):
    M = np.zeros((n, n), dtype=np.uint8)
    for i, j in enumerate(perm):
        M[i, j] = 1
    return M

def _apply_bits(M, idx, n=16):
    idx = np.asarray(idx)
    out = np.zeros_like(idx)
    for i in range(n):
        acc = np.zeros_like(idx)
        for j in range(n):
            if M[i, j]:
                acc ^= (idx >> (n - 1 - j)) & 1
        out |= acc << (n - 1 - i)
    return out

def _ring_matrix(jump):
    M = np.eye(N, dtype=np.uint8)
    for k in range(N):
        E = np.eye(N, dtype=np.uint8)
        E[(k + jump) % N, k] = 1
        M = (E @ M) % 2
    return M

TAU = _perm_mat([9, 10, 11, 12, 13, 14, 15, 7, 8, 0, 1, 2, 3, 4, 5, 6])
SIG78 = _perm_mat([0, 1, 2, 3, 4, 5, 6, 8, 7, 9, 10, 11, 12, 13, 14, 15])

# ----------------------------------------------------------------------------
# plan construction (gate scheduling; batch independent except amp diagonals)
# ----------------------------------------------------------------------------
def _rx(th):
    c, s = np.cos(th / 2), np.sin(th / 2)
    return np.array([[c, -1j * s], [-1j * s, c]])

def _ry(th):
    c, s = np.cos(th / 2), np.sin(th / 2)
    return np.array([[c, -s], [s, c]])

def _rz(th):
    return np.array([[np.exp(-0.5j * th), 0], [0, np.exp(0.5j * th)]])

def _embed(M8, pos):
    E = np.eye(N, dtype=np.uint8)
    for i in range(8):
        for j in range(8):
            E[pos[i], pos[j]] = M8[i, j]
    return E

def _transvection(targets, source):
    E = np.eye(N, dtype=np.uint8)
    for t in targets:
        E[t, source] ^= 1
    return E

def _gen_gate_8(d16, r16, U):
    d8 = 0
    r8 = 0
    for j in range(8):
        d8 |= int(d16[j]) << (7 - j)
        r8 |= int(r16[j]) << (7 - j)
    O = np.zeros((256, 256), dtype=np.complex128)
    g = np.arange(256)
    c = np.zeros(256, dtype=np.int64)
    for b in range(8):
        if (r8 >> b) & 1:
            c ^= (g >> b) & 1
    O[g, g] = U[c, c]
    O[g ^ d8, g] = np.asarray(U)[c ^ 1, c]
    return O

def _perm8_op(Lam8):
    O = np.zeros((256, 256))
    idx = np.arange(256)
    new = np.zeros_like(idx)
    for i in range(8):
        acc = np.zeros_like(idx)
        for j in range(8):
            if Lam8[i, j]:
                acc ^= (idx >> (7 - j)) & 1
        new |= acc << (7 - i)
    O[new, idx] = 1.0
    return O

def build_plan(layer_params):
    M1 = _ring_matrix(1)
    M2 = _ring_matrix(2)
    M1i = _inv(M1)
    A = list(range(8))
    B = list(range(8, 16))
    K = M1[np.ix_(A, A)]
    Nb = M1[np.ix_(B, B)]
    Ki = _inv(K)
    S = (Nb + _mm(M1[np.ix_(B, A)], Ki, M1[np.ix_(A, B)])) % 2

    U1 = [_rz(layer_params[0, 2, w]) @ _ry(layer_params[0, 1, w]) @ _rx(layer_params[0, 0, w]) for w in range(N)]
    U2 = [_ry(layer_params[1, 1, w]) @ _rx(layer_params[1, 0, w]) for w in range(N)]

    Phi = np.eye(N, dtype=np.uint8)
    stages = []

    def gate_op(w, U, Phi):
        d = Phi[:, w].copy()
        r = _inv(Phi).T[:, w].copy()
        assert not d[8:].any() and not r[8:].any(), f"gate support escape w={w}"
        return _gen_gate_8(d, r, U)

    # Stage 1: layer-1 A gates + fold K
    O8 = np.eye(256, dtype=np.complex128)
    for w in A:
        O8 = gate_op(w, U1[w], Phi) @ O8
    E = _embed(K, list(range(8)))
    LK = _embed(K, list(range(8)))
    O8 = _perm8_op(K) @ O8
    Phi = _mm(E, Phi, _inv(LK))
    stages.append(dict(O8=O8))
    Phi = _mm(TAU, SIG78, Phi)

    # Stage 2: layer-1 B gates + fold S (position-rep) + out-XOR T_up'
    O8 = np.eye(256, dtype=np.complex128)
    for w in B:
        O8 = gate_op(w, U1[w], Phi) @ O8
    posmap = [7, 0, 1, 2, 3, 4, 5, 6]
    S8pos = np.zeros((8, 8), np.uint8)
    for dnew in range(8):
        for dold in range(8):
            if S[dnew, dold]:
                S8pos[dnew, posmap[dold]] ^= 1
    E = _embed(S8pos, list(range(8)))
    LS = _embed(S, list(range(8, 16)))
    O8 = _perm8_op(S8pos) @ O8
    Phi = _mm(E, Phi, _inv(LS))
    Edev = _transvection([9], 7)
    Ltgt = _transvection([0], 15)
    Phi = _mm(Edev, Phi, _inv(Ltgt))
    stages.append(dict(O8=O8))
    Phi = _mm(TAU, SIG78, Phi)

    # Stage 3: in-XOR T_low + layer-2 A gates
    Edev = _transvection(list(range(8, 16)), 7)
    Ltgt = _transvection(list(range(8, 16)), 7)
    Phi = _mm(Edev, Phi, _inv(Ltgt))
    O8 = np.eye(256, dtype=np.complex128)
    for w in A:
        O8 = gate_op(w, U2[w], Phi) @ O8
    stages.append(dict(O8=O8))
    Phi = _mm(TAU, SIG78, Phi)

    # Stage 4: layer-2 B gates
    O8 = np.eye(256, dtype=np.complex128)
    for w in B:
        O8 = gate_op(w, U2[w], Phi) @ O8
    stages.append(dict(O8=O8))

    Zmat = _mm(Phi, _inv(M2))
    gather = _apply_bits(Zmat, np.arange(65536))
    return stages, gather

def amp_diags(nf_one):
    alpha = nf_one.reshape(16, 3).sum(axis=1)
    v = np.stack([np.cos(alpha / 2), np.sin(alpha / 2)], axis=1)
    g = np.arange(256)
    ampA = np.ones(256)
    for w in range(8):
        ampA = ampA * v[w, (g >> (7 - w)) & 1]
    ampB = np.ones(256)
    ampB = ampB * v[8, g & 1]
    for k in range(7):
        ampB = ampB * v[9 + k, (g >> (7 - k)) & 1]
    return ampA, ampB

def stat_blocks(O8):
    """[12, 128, 128] f32: for h in 0,1: for fi in 0,1: (reT, imT, -imT)."""
    T = O8.reshape(128, 2, 128, 2)
    out = np.empty((12, 128, 128), np.float32)
    i = 0
    for h in (0, 1):
        for fi in (0, 1):
            blk = T[:, h, :, fi]
            out[i] = blk.real.T.astype(np.float32); i += 1
            out[i] = blk.imag.T.astype(np.float32); i += 1
            out[i] = (-blk.imag.T).astype(np.float32); i += 1
    return out

def phase_tables(ef_one):
    W = np.zeros((16, 16))
    idx = 1
    for q in range(16):
        for e in range(q + 1, 16):
            W[q, e] = ef_one[idx]
            idx += 1
    p = np.arange(128)
    f = np.arange(512)
    pbits = ((p[:, None] >> (6 - np.arange(7))[None, :]) & 1).astype(np.float64)  # [128,7]
    fbits = ((f[:, None] >> (15 - np.arange(7, 16))[None, :]) & 1).astype(np.float64)  # [512,9]
    phL = np.zeros((9, 128), np.float32)
    phR = np.zeros((9, 512), np.float32)
    phL[0:7] = pbits.T
    phR[0:7] = 0.5 * (W[0:7, 7:16] @ (2 * fbits.T - 1))
    # phi_pp: pairs within wires 0..6
    Wpp = W[0:7, 0:7]
    phipp = 0.5 * np.einsum('pq,qe,pe->p', pbits, Wpp, 2 * pbits - 1)
    phL[7] = phipp
    phR[7] = 1.0
    phL[8] = 1.0
    Wff = W[7:16, 7:16]
    phiff = 0.5 * np.einsum('pq,qe,pe->p', fbits, Wff, 2 * fbits - 1)
    phR[8] = phiff
    return phL, phR

# ----------------------------------------------------------------------------
# device program (built once per process)
# ----------------------------------------------------------------------------
_NC_CACHE = {}

def build_device_program():
    if "nc" in _NC_CACHE:
        return _NC_CACHE["nc"]
    nc = bass.Bass()
    phL = nc.declare_dram_parameter("phL", [9, 128], F32, isOutput=False)
    phR = nc.declare_dram_parameter("phR", [9, 512], F32, isOutput=False)
    stp = [nc.declare_dram_parameter(f"st{s}", [12, 128, 128], F32R, isOutput=False)
           for s in range(4)]
    probs_out = nc.declare_dram_parameter("probs", [128, 512], F32, isOutput=True)

    CHUNK_SRC = [0, 2, 1, 3]  # sigma78: transpose chunk c reads source chunk

    with tile.TileContext(nc) as tc:
        with tc.tile_pool(name="sb", bufs=1) as sb, \
             tc.tile_pool(name="ps", bufs=1, space="PSUM") as ps:
            # --- input DMAs ---
            stats_sb = []
            for s in range(4):
                t = sb.tile([128, 12 * 128], F32R, name=f"stats{s}")
                nc.sync.dma_start(t[:].rearrange("p (n c) -> p n c", n=12),
                                  stp[s][:].transpose([1, 0, 2]))
                stats_sb.append(t)
            phL_sb = sb.tile([9, 128], F32, name="phL_sb")
            phR_sb = sb.tile([9, 512], F32, name="phR_sb")
            nc.sync.dma_start(phL_sb[:], phL[:])
            nc.sync.dma_start(phR_sb[:], phR[:])

            ident = sb.tile([128, 128], F32R, name="ident")
            identf = sb.tile([128, 128], F32, name="identf")
            nc.gpsimd.memset(identf[:], 0.0)
            nc.gpsimd.affine_select(
                out=identf[:], in_=identf[:],
                compare_op=ALU.not_equal, fill=1.0, base=0,
                pattern=[[-1, 128]], channel_multiplier=1)
            nc.vector.tensor_copy(out=ident[:], in_=identf[:])
            pi2b = sb.tile([128, 1], F32, name="pi2b")
            nc.gpsimd.memset(pi2b[:], PI / 2)

            # --- phi field ---
            psum_phi = ps.tile([128, 512], F32, name="psum_phi")
            nc.tensor.matmul(psum_phi[:], phL_sb[:], phR_sb[:], start=True, stop=True)

            # --- sin/cos with range reduction ---
            def trig(out_tile, bias_ap, quarter):
                t_s = sb.tile([128, 512], F32, name=f"t_s{quarter}")
                if quarter:
                    nc.vector.tensor_scalar(out=t_s[:], in0=psum_phi[:],
                                            scalar1=1.0 / (2 * PI), scalar2=0.25,
                                            op0=ALU.mult, op1=ALU.add)
                else:
                    nc.vector.tensor_scalar(out=t_s[:], in0=psum_phi[:],
                                            scalar1=1.0 / (2 * PI), scalar2=None,
                                            op0=ALU.mult)
                k_i = sb.tile([128, 512], I32, name=f"k_i{quarter}")
                nc.vector.tensor_copy(out=k_i[:], in_=t_s[:])
                k_f = sb.tile([128, 512], F32, name=f"k_f{quarter}")
                nc.vector.tensor_copy(out=k_f[:], in_=k_i[:])
                red = sb.tile([128, 512], F32, name=f"red{quarter}")
                nc.vector.scalar_tensor_tensor(out=red[:], in0=k_f[:], scalar=-2 * PI,
                                               in1=psum_phi[:], op0=ALU.mult, op1=ALU.add)
                if bias_ap is None:
                    nc.scalar.activation(out_tile[:], red[:], ACT.Sin)
                else:
                    nc.scalar.activation(out_tile[:], red[:], ACT.Sin, bias=bias_ap)

            cur_im = sb.tile([128, 512], F32R, name="state_im0")
            trig(cur_im, None, 0)
            cur_re = sb.tile([128, 512], F32R, name="state_re0")
            trig(cur_re, pi2b[:], 1)

            # --- 4 matmul stages ---
            for s in range(4):
                mm_re = ps.tile([128, 512], F32, name=f"mm_re{s}", tag="mm_re")
                mm_im = ps.tile([128, 512], F32, name=f"mm_im{s}", tag="mm_im")

                def stat(idx):
                    return stats_sb[s][:, idx * 128:(idx + 1) * 128]

                def rhs_ap(plane, fi):
                    base = plane[:, fi * 256:(fi + 1) * 256]
                    if s == 2 and fi == 1:
                        return plane[:, 256:512][:, ::-1]
                    return base

                def out_ap(psum, h):
                    if s == 1 and h == 1:
                        v = psum[:, 256:512].rearrange("p (a b c) -> p a b c",
                                                       a=2, b=2, c=64)
                        return v[:, :, ::-1, :]
                    return psum[:, h * 256:(h + 1) * 256]

                for h in (0, 1):
                    # out_re[h] = reT[h,0]@re0 + reT[h,1]@re1 - imT[h,0]@im0 - imT[h,1]@im1
                    ops = [(stat(3 * (2 * h + 0) + 0), rhs_ap(cur_re, 0)),
                           (stat(3 * (2 * h + 1) + 0), rhs_ap(cur_re, 1)),
                           (stat(3 * (2 * h + 0) + 2), rhs_ap(cur_im, 0)),
                           (stat(3 * (2 * h + 1) + 2), rhs_ap(cur_im, 1))]
                    for i, (w_ap, x_ap) in enumerate(ops):
                        nc.tensor.matmul(out_ap(mm_re, h), w_ap, x_ap,
                                         start=(i == 0), stop=(i == len(ops) - 1))
                    # out_im[h] = imT[h,0]@re0 + imT[h,1]@re1 + reT[h,0]@im0 + reT[h,1]@im1
                    ops = [(stat(3 * (2 * h + 0) + 1), rhs_ap(cur_re, 0)),
                           (stat(3 * (2 * h + 1) + 1), rhs_ap(cur_re, 1)),
                           (stat(3 * (2 * h + 0) + 0), rhs_ap(cur_im, 0)),
                           (stat(3 * (2 * h + 1) + 0), rhs_ap(cur_im, 1))]
                    for i, (w_ap, x_ap) in enumerate(ops):
                        nc.tensor.matmul(out_ap(mm_im, h), w_ap, x_ap,
                                         start=(i == 0), stop=(i == len(ops) - 1))

                if s < 3:
                    tr_re = sb.tile([128, 512], F32, name=f"tr_re{s}", tag="tr_re")
                    tr_im = sb.tile([128, 512], F32, name=f"tr_im{s}", tag="tr_im")
                    nc.scalar.copy(tr_re[:], mm_re[:])
                    nc.vector.tensor_copy(out=tr_im[:], in_=mm_im[:])
                    pt_re = ps.tile([128, 512], F32, name=f"pt_re{s}", tag="pt_re")
                    pt_im = ps.tile([128, 512], F32, name=f"pt_im{s}", tag="pt_im")
                    for c in range(4):
                        sc = CHUNK_SRC[c]
                        nc.tensor.transpose(pt_re[:, c * 128:(c + 1) * 128],
                                            tr_re[:, sc * 128:(sc + 1) * 128], identf[:])
                        nc.tensor.transpose(pt_im[:, c * 128:(c + 1) * 128],
                                            tr_im[:, sc * 128:(sc + 1) * 128], identf[:])
                    cur_re = sb.tile([128, 512], F32R, name=f"state_re{s + 1}", tag="state_re")
                    cur_im = sb.tile([128, 512], F32R, name=f"state_im{s + 1}", tag="state_im")
                    nc.vector.tensor_copy(out=cur_re[:], in_=pt_re[:])
                    nc.scalar.copy(cur_im[:], pt_im[:])
                else:
                    q1 = sb.tile([128, 512], F32, name="q1")
                    nc.scalar.activation(q1[:], mm_re[:], ACT.Square)
                    q2 = sb.tile([128, 512], F32, name="q2")
                    nc.scalar.activation(q2[:], mm_im[:], ACT.Square)
                    q3 = sb.tile([128, 512], F32, name="q3")
                    nc.vector.tensor_tensor(out=q3[:], in0=q1[:], in1=q2[:],
                                            op=ALU.add)
                    nc.sync.dma_start(probs_out[:], q3[:])

    _NC_CACHE["nc"] = nc
    return nc

# ----------------------------------------------------------------------------
# entry point
# ----------------------------------------------------------------------------
_PLAN_CACHE = {}

def kernel(node_inputs, node_indices, edge_inputs, layer_params, _want_trace=False):
    node_inputs = np.asarray(node_inputs, np.float32)
    edge_inputs = np.asarray(edge_inputs, np.float32)
    layer_params = np.asarray(layer_params, np.float32)
    B = node_inputs.shape[0]
    assert B == 8

    key = layer_params.tobytes()
    if _PLAN_CACHE.get("key") != key:
        stages, gather = build_plan(layer_params.astype(np.float64))
        _PLAN_CACHE.update(key=key, stages=stages, gather=gather,
                           st2=stat_blocks(stages[2]["O8"]),
                           st3=stat_blocks(stages[3]["O8"]))
    stages = _PLAN_CACHE["stages"]
    gather = _PLAN_CACHE["gather"]

    in_maps = []
    for b in range(B):
        ampA, ampB = amp_diags(node_inputs[b].astype(np.float64))
        st0 = stat_blocks(stages[0]["O8"] @ np.diag(ampA))
        st1 = stat_blocks(stages[1]["O8"] @ np.diag(ampB))
        phL, phR = phase_tables(edge_inputs[b].astype(np.float64))
        in_maps.append(dict(phL=phL, phR=phR, st0=st0, st1=st1,
                            st2=_PLAN_CACHE["st2"], st3=_PLAN_CACHE["st3"]))

    nc = build_device_program()
    res = run_bass_kernel_spmd(nc, in_maps, list(range(8)), trace=_want_trace)

    out = np.empty((B, 65536), np.float32)
    for b in range(B):
        flat = res.results[b]["probs"].reshape(-1)
        out[b] = flat[gather]
    if _want_trace:
        return out, res
    return out
